# revision 9
# baseline (speedup 1.0000x reference)
"""Multi-head attention (B=4, S=2048, D=1024, H=16, dk=64) on 8 TRN2 NeuronCores.

Sharding: core = (batch b, head-group g) with b in 0..3, g in 0..1.
Each core projects q/k/v for its batch with its 512 Wq columns (8 heads),
runs attention for those heads, and applies its 512 Wo rows, producing a
partial (transposed) output. Host sums the two half-head partials per batch.

All device compute in bf16 (fp32 PSUM accumulation). Layout is transposed
(feature-major) so every matmul has its contraction dim on partitions:
  qT,kT: [dh, S]   v: [S, dh(+ones col)]   scores^T: [j, i]   out^T: [e, i]

Per head pair, the two K=64 score matmuls are packed into the PE array via
base-partition-derived tile_position (rows 0-63 / 64-127 concurrently).
The attn@v matmul carries a 65th ones-column of v so each PSUM tile's last
partition accumulates the softmax denominator for free.

Software pipelining: kT is projected chunk-wise so attention i=0 starts
early; v is projected inside (i=0, pair=0)'s j-loop; the output projection
of i-1 and the q projection of i+1 are spread between pair blocks so the
scalar engine (exp) stays saturated across i boundaries.
"""

import numpy as np
import ml_dtypes

B, S, D = 4, 2048, 1024
N_HEAD, DK = 16, 64
P = 128
DH = 512          # head-group width (8 heads)
PAIRS = 4         # head pairs per core
C = D // P        # contraction chunks for projections (8)
IT = S // 512     # i tiles of 512 (4)
JT = S // P       # j tiles of 128 (16)
ET = D // P       # output-row tiles (8)

BF16 = ml_dtypes.bfloat16

_CACHE = {}


def _build(has_bq: bool):
    import concourse.bacc as bacc
    import concourse.mybir as mybir
    import concourse.tile as tile

    nc = bacc.Bacc()
    dt = mybir.dt
    AF = mybir.ActivationFunctionType

    qt_d = nc.dram_tensor("qt", [D, S], dt.bfloat16, kind="ExternalInput")
    kt_d = nc.dram_tensor("kt", [D, S], dt.bfloat16, kind="ExternalInput")
    vt_d = nc.dram_tensor("vt", [D, S], dt.bfloat16, kind="ExternalInput")
    wq_d = nc.dram_tensor("wq", [D, DH], dt.bfloat16, kind="ExternalInput")
    wo_d = nc.dram_tensor("wo", [DH, D], dt.bfloat16, kind="ExternalInput")
    if has_bq:
        bq_d = nc.dram_tensor("bq", [1, DH], dt.bfloat16, kind="ExternalInput")
    out_d = nc.dram_tensor("out", [D, S], dt.float32, kind="ExternalOutput")

    with tile.TileContext(nc) as tc:
        with (
            tc.tile_pool(name="consts", bufs=1) as consts,
            tc.tile_pool(name="xin", bufs=2) as xin,
            tc.tile_pool(name="kin", bufs=4) as kin,
            tc.tile_pool(name="proj_out", bufs=1) as proj_out,
            tc.tile_pool(name="expp", bufs=3) as expp,
            tc.tile_pool(name="osb", bufs=4) as osbp,
            tc.tile_pool(name="small", bufs=4) as small,
            tc.tile_pool(name="outsb", bufs=3) as outsb,
            tc.tile_pool(name="mm_ps", bufs=2, space="PSUM") as mm_ps,
            tc.tile_pool(name="sc_ps", bufs=2, space="PSUM") as sc_ps,
            tc.tile_pool(name="o_ps", bufs=1, space="PSUM") as o_ps,
        ):
            wq_sb = consts.tile([P, C, DH], dt.bfloat16)
            nc.sync.dma_start(out=wq_sb[:], in_=wq_d.rearrange("(c p) d -> p c d", p=P))
            wo_sb = consts.tile([P, PAIRS, D], dt.bfloat16)
            nc.sync.dma_start(out=wo_sb[:], in_=wo_d.rearrange("(c p) e -> p c e", p=P))
            if has_bq:
                bq_sb = consts.tile([1, DH], dt.bfloat16)
                nc.sync.dma_start(out=bq_sb[:], in_=bq_d[:])
                ones_sb = consts.tile([1, 512], dt.bfloat16)
                nc.vector.memset(ones_sb[:], 1.0)

            qT_sb = proj_out.tile([P, PAIRS, S], dt.bfloat16)
            kT_sb = proj_out.tile([P, PAIRS, S], dt.bfloat16)
            # v in natural layout; per head 65 columns: 64 of v plus a ones
            # column that makes the attn@v matmul also emit the softmax denom.
            v_sb = proj_out.tile([P, JT, PAIRS * 2, DK + 1], dt.bfloat16)
            nc.vector.memset(v_sb[:, :, :, DK : DK + 1], 1.0)
            outt_sb = proj_out.tile([P, PAIRS, S], dt.bfloat16)

            vt_r = vt_d.rearrange("(c p) s -> p c s", p=P)
            kt_r = kt_d.rearrange("(c p) s -> p c s", p=P)
            qt_r = qt_d.rearrange("(c p) s -> p c s", p=P)

            def proj_step(dst_sb, lhs_chunks, rhs_chunks, bias_lhs, bias_rhs):
                """One [128, 512] projection psum: accumulate over C chunks."""
                ps = mm_ps.tile([P, 512], dt.float32, tag="ps", name="ps")
                for c in range(C):
                    nc.tensor.matmul(
                        ps[:], lhs_chunks(c), rhs_chunks(c),
                        start=(c == 0), stop=(c == C - 1 and not has_bq),
                    )
                if has_bq:
                    nc.tensor.matmul(ps[:], bias_lhs(), bias_rhs(),
                                     start=False, stop=True)
                return ps

            def qproj(i, d, qt_in):
                isl = slice(i * 512, (i + 1) * 512)
                dsl = slice(d * P, (d + 1) * P)
                ps = proj_step(
                    qT_sb,
                    lambda c: wq_sb[:, c, dsl],
                    lambda c, qt_in=qt_in: qt_in[:, c, :],
                    lambda: bq_sb[0:1, dsl],
                    lambda: ones_sb[0:1, :],
                )
                nc.vector.tensor_copy(out=qT_sb[:, d, isl], in_=ps[:])

            def fin(i, e):
                isl = slice(i * 512, (i + 1) * 512)
                esl = slice(e * P, (e + 1) * P)
                ps = mm_ps.tile([P, 512], dt.float32, tag="ps", name="ps")
                for hc in range(PAIRS):
                    nc.tensor.matmul(
                        ps[:], wo_sb[:, hc, esl], outt_sb[:, hc, isl],
                        start=(hc == 0), stop=(hc == PAIRS - 1),
                    )
                of = outsb.tile([P, 512], dt.float32, tag="of", name="of")
                nc.vector.tensor_copy(out=of[:], in_=ps[:])
                nc.sync.dma_start(out=out_d[esl, isl], in_=of[:])

            # ---- prologue: kT (chunk-wise) and qT(i=0) ----
            kt_ins = []
            for ii in range(IT):
                t = kin.tile([P, C, 512], dt.bfloat16, tag="kin", name="kt_in")
                nc.sync.dma_start(
                    out=t[:], in_=kt_r[:, :, ii * 512 : (ii + 1) * 512]
                )
                kt_ins.append(t)
            qt_in0 = xin.tile([P, C, 512], dt.bfloat16, tag="qin", name="qt_in")
            nc.sync.dma_start(out=qt_in0[:], in_=qt_r[:, :, 0:512])
            qt_in_next = {0: qt_in0}

            def kproj(d, ii):
                isl = slice(ii * 512, (ii + 1) * 512)
                dsl = slice(d * P, (d + 1) * P)
                ps = proj_step(
                    kT_sb,
                    lambda c: wq_sb[:, c, dsl],
                    lambda c: kt_ins[ii][:, c, :],
                    lambda: bq_sb[0:1, dsl],
                    lambda: ones_sb[0:1, :],
                )
                nc.vector.tensor_copy(out=kT_sb[:, d, isl], in_=ps[:])

            def qproj_filler(i, d):
                def _f():
                    if i not in qt_in_next:
                        t = xin.tile([P, C, 512], dt.bfloat16, tag="qin", name="qt_in")
                        nc.sync.dma_start(
                            out=t[:], in_=qt_r[:, :, i * 512 : (i + 1) * 512]
                        )
                        qt_in_next[i] = t
                    qproj(i, d, qt_in_next[i])
                return _f

            # only k/q chunk 0 before attention starts; the rest rides as
            # filler work inside the attention j-loops
            for ii in range(IT):
                kproj(0, ii)
            qproj(0, 0, qt_in0)

            # filler queues: work to sprinkle into PE slack of the ACT-bound
            # attention j-loops. Pair pr of tile i consumes 5 slots; the
            # ordering guarantees kT/qT chunk pr+1 completes before pair pr+1.
            fillers = {0: [], 1: [], 2: [], 3: []}
            for d in range(1, PAIRS):
                fillers[0].append(qproj_filler(0, d))
                for ii in range(IT):
                    fillers[0].append(lambda d=d, ii=ii: kproj(d, ii))
            for d in range(PAIRS):
                fillers[0].append(qproj_filler(1, d))
            for i in (1, 2, 3):
                for e in range(ET):
                    fillers[i].append(lambda i=i, e=e: fin(i - 1, e))
                if i < IT - 1:
                    for d in range(PAIRS):
                        fillers[i].append(qproj_filler(i + 1, d))

            FILLER_JS = (3, 6, 9, 12, 15)

            # ---- main loop over i tiles ----
            for i in range(IT):
                isl = slice(i * 512, (i + 1) * 512)
                for pr in range(PAIRS):
                    oa = o_ps.tile([DK + 1, 512], dt.float32, tag="oa", name="oa")
                    ob = o_ps.tile([DK + 1, 512], dt.float32, tag="ob", name="ob")
                    for j in range(JT):
                        jsl = slice(j * P, (j + 1) * P)
                        if i == 0 and pr == 0:
                            # v projection rides inside the first j-loop
                            if j % 4 == 0:
                                ic = j // 4
                                vt_in = xin.tile(
                                    [P, C, 512], dt.bfloat16, tag="vin", name="vt_in"
                                )
                                nc.sync.dma_start(
                                    out=vt_in[:],
                                    in_=vt_r[:, :, ic * 512 : (ic + 1) * 512],
                                )
                            jl = j % 4
                            ps = proj_step(
                                v_sb,
                                lambda c, vt_in=vt_in, jl=jl: vt_in[:, c, jl * P : (jl + 1) * P],
                                lambda c: wq_sb[:, c, :],
                                lambda: ones_sb[0:1, 0:P],
                                lambda: bq_sb[0:1, :],
                            )
                            nc.vector.tensor_copy(
                                out=v_sb[:, j, :, 0:DK],
                                in_=ps[:].rearrange("p (h k) -> p h k", h=PAIRS * 2),
                            )
                        sc = sc_ps.tile([P, 2, 512], dt.float32, tag="sc", name="sc")
                        # two heads (K=64 each) packed into the PE array
                        nc.tensor.matmul(
                            sc[:, 0, :], kT_sb[0:DK, pr, jsl], qT_sb[0:DK, pr, isl],
                            start=True, stop=True,
                        )
                        nc.tensor.matmul(
                            sc[:, 1, :], kT_sb[DK:P, pr, jsl], qT_sb[DK:P, pr, isl],
                            start=True, stop=True,
                        )
                        ex = expp.tile([P, 2, 512], dt.bfloat16, tag="ex", name="ex")
                        nc.scalar.activation(
                            out=ex[:], in_=sc[:], func=AF.Exp, scale=0.125
                        )
                        nc.tensor.matmul(
                            oa[:], v_sb[:, j, 2 * pr, :], ex[:, 0, :],
                            start=(j == 0), stop=(j == JT - 1),
                        )
                        nc.tensor.matmul(
                            ob[:], v_sb[:, j, 2 * pr + 1, :], ex[:, 1, :],
                            start=(j == 0), stop=(j == JT - 1),
                        )
                        if j in FILLER_JS and fillers[i]:
                            fillers[i].pop(0)()
                    # normalize: row DK of each psum is the softmax denominator.
                    # Single f32 copy frees the PSUM bank; the reciprocal chain
                    # then runs off the PE critical path.
                    for o_ps_t, upper in ((oa, False), (ob, True)):
                        o_sb = osbp.tile([DK + 1, 512], dt.float32, tag="osb", name="o_sb")
                        nc.vector.tensor_copy(out=o_sb[:], in_=o_ps_t[:])
                        den0 = small.tile([1, 512], dt.float32, tag="den0", name="den0")
                        nc.sync.dma_start(out=den0[:], in_=o_sb[DK : DK + 1, :])
                        bc = small.tile([DK, 512], dt.float32, tag="bc", name="bc")
                        nc.gpsimd.partition_broadcast(bc[:], den0[:])
                        rc = small.tile([DK, 512], dt.float32, tag="rc", name="rc")
                        nc.vector.reciprocal_approx_fast(out=rc[:], in_=bc[:])
                        if not upper:
                            nc.vector.tensor_mul(
                                out=outt_sb[0:DK, pr, isl],
                                in0=o_sb[0:DK, :],
                                in1=rc[:],
                            )
                        else:
                            tmp = small.tile([DK, 512], dt.bfloat16, tag="tmp", name="tmp")
                            nc.vector.tensor_mul(
                                out=tmp[:], in0=o_sb[0:DK, :], in1=rc[:]
                            )
                            nc.sync.dma_start(out=outt_sb[DK:P, pr, isl], in_=tmp[:])

                # drain any fillers not consumed inside the j-loops
                while fillers[i]:
                    fillers[i].pop(0)()

            # ---- epilogue: output projection of the last i tile ----
            for e in range(ET):
                fin(IT - 1, e)

    nc.compile()
    return nc


def _get_program(has_bq: bool):
    key = ("prog", has_bq)
    if key not in _CACHE:
        _CACHE[key] = _build(has_bq)
    return _CACHE[key]


def _run(inputs, trace=False):
    from concourse.bass_utils import run_bass_kernel_spmd

    Q = np.asarray(inputs["Q"], dtype=np.float32)
    K = np.asarray(inputs["K"], dtype=np.float32)
    V = np.asarray(inputs["V"], dtype=np.float32)
    Wq = np.asarray(inputs["Wq"], dtype=np.float32)
    bq = np.asarray(inputs["bq"], dtype=np.float32)
    Wo = np.asarray(inputs["Wo"], dtype=np.float32)
    bo = np.asarray(inputs["bo"], dtype=np.float32)

    has_bq = bool(np.any(bq))
    nc = _get_program(has_bq)

    qt = [np.ascontiguousarray(Q[b].T).astype(BF16) for b in range(B)]
    kt = [np.ascontiguousarray(K[b].T).astype(BF16) for b in range(B)]
    vt = [np.ascontiguousarray(V[b].T).astype(BF16) for b in range(B)]
    wq = [np.ascontiguousarray(Wq[:, g * DH : (g + 1) * DH]).astype(BF16) for g in range(2)]
    wo = [np.ascontiguousarray(Wo[g * DH : (g + 1) * DH, :]).astype(BF16) for g in range(2)]
    bqs = [bq[None, g * DH : (g + 1) * DH].astype(BF16) for g in range(2)]

    in_maps = []
    for b in range(B):
        for g in range(2):
            m = {"qt": qt[b], "kt": kt[b], "vt": vt[b], "wq": wq[g], "wo": wo[g]}
            if has_bq:
                m["bq"] = bqs[g]
            in_maps.append(m)

    res = run_bass_kernel_spmd(nc, in_maps, core_ids=list(range(8)), trace=trace)

    out = np.empty((B, S, D), np.float32)
    for b in range(B):
        part = res.results[2 * b]["out"] + res.results[2 * b + 1]["out"]
        out[b] = part.T + bo
    return out, res


def kernel(**inputs) -> np.ndarray:
    return _run(inputs, trace=False)[0]


def _ensure_ntff_hook():
    """The agent image's antenv lacks axon_hooks; synthesize it so
    run_bass_kernel_spmd(trace=True) can capture NTFF profiles."""
    import sys, types
    try:
        from antenv.axon_hooks import get_axon_ntff_profile_hook  # noqa: F401
        return
    except ImportError:
        pass
    mod = types.ModuleType("antenv.axon_hooks")
    mod._hook = None
    mod.set_axon_ntff_profile_hook = lambda h: setattr(mod, "_hook", h)
    mod.get_axon_ntff_profile_hook = lambda: mod._hook
    sys.modules["antenv.axon_hooks"] = mod
    import antenv
    antenv.axon_hooks = mod
    try:
        from trn_agent_boot.trn_boot import _ntff_profile_via_ctypes
        mod._hook = _ntff_profile_via_ctypes("/opt/axon/libaxon_pjrt.so")
    except Exception as e:  # degrade: trace skipped, run still works
        print(f"ntff hook install failed: {e}")


def kernel_traced(**inputs):
    _ensure_ntff_hook()
    return _run(inputs, trace=True)


# revision 17
# speedup vs baseline: 1.0037x; 1.0037x over previous
"""Multi-head attention (B=4, S=2048, D=1024, H=16, dk=64) on 8 TRN2 NeuronCores.

Sharding: core = (batch b, head-group g) with b in 0..3, g in 0..1.
Each core projects q/k/v for its batch with its 512 Wq columns (8 heads),
runs attention for those heads, and applies its 512 Wo rows, producing a
partial (transposed) output. Host sums the two half-head partials per batch.

All device compute in bf16 (fp32 PSUM accumulation). Layout is transposed
(feature-major) so every matmul has its contraction dim on partitions:
  qT,kT: [dh, S]   v: [S, dh(+ones col)]   scores^T: [j, i]   out^T: [e, i]

Per head pair, the two K=64 score matmuls are packed into the PE array via
base-partition-derived tile_position (rows 0-63 / 64-127 concurrently).
The attn@v matmul carries a 65th ones-column of v so each PSUM tile's last
partition accumulates the softmax denominator for free.

Software pipelining: kT is projected chunk-wise so attention i=0 starts
early; v is projected inside (i=0, pair=0)'s j-loop; the output projection
of i-1 and the q projection of i+1 are spread between pair blocks so the
scalar engine (exp) stays saturated across i boundaries.
"""

import numpy as np
import ml_dtypes

B, S, D = 4, 2048, 1024
N_HEAD, DK = 16, 64
P = 128
DH = 512          # head-group width (8 heads)
PAIRS = 4         # head pairs per core
C = D // P        # contraction chunks for projections (8)
IT = S // 512     # i tiles of 512 (4)
JT = S // P       # j tiles of 128 (16)
ET = D // P       # output-row tiles (8)

BF16 = ml_dtypes.bfloat16

_CACHE = {}


def _build(has_bq: bool):
    import concourse.bacc as bacc
    import concourse.mybir as mybir
    import concourse.tile as tile

    nc = bacc.Bacc()
    dt = mybir.dt
    AF = mybir.ActivationFunctionType

    qt_d = nc.dram_tensor("qt", [D, S], dt.bfloat16, kind="ExternalInput")
    kt_d = nc.dram_tensor("kt", [D, S], dt.bfloat16, kind="ExternalInput")
    vt_d = nc.dram_tensor("vt", [D, S], dt.bfloat16, kind="ExternalInput")
    wq_d = nc.dram_tensor("wq", [D, DH], dt.bfloat16, kind="ExternalInput")
    wo_d = nc.dram_tensor("wo", [DH, D], dt.bfloat16, kind="ExternalInput")
    if has_bq:
        bq_d = nc.dram_tensor("bq", [1, DH], dt.bfloat16, kind="ExternalInput")
    out_d = nc.dram_tensor("out", [D, S], dt.float32, kind="ExternalOutput")

    with tile.TileContext(nc) as tc:
        with (
            tc.tile_pool(name="consts", bufs=1) as consts,
            tc.tile_pool(name="xin", bufs=2) as xin,
            tc.tile_pool(name="kin", bufs=4) as kin,
            tc.tile_pool(name="proj_out", bufs=1) as proj_out,
            tc.tile_pool(name="expp", bufs=3) as expp,
            tc.tile_pool(name="osb", bufs=4) as osbp,
            tc.tile_pool(name="small", bufs=4) as small,
            tc.tile_pool(name="outsb", bufs=3) as outsb,
            tc.tile_pool(name="mm_ps", bufs=2, space="PSUM") as mm_ps,
            tc.tile_pool(name="sc_ps", bufs=2, space="PSUM") as sc_ps,
            tc.tile_pool(name="o_ps", bufs=1, space="PSUM") as o_ps,
        ):
            wq_sb = consts.tile([P, C, DH], dt.bfloat16)
            nc.sync.dma_start(out=wq_sb[:], in_=wq_d.rearrange("(c p) d -> p c d", p=P))
            if has_bq:
                bq_sb = consts.tile([1, DH], dt.bfloat16)
                nc.sync.dma_start(out=bq_sb[:], in_=bq_d[:])
                ones_sb = consts.tile([1, 512], dt.bfloat16)
                nc.vector.memset(ones_sb[:], 1.0)

            qT_sb = proj_out.tile([P, PAIRS, S], dt.bfloat16)
            kT_sb = proj_out.tile([P, PAIRS, S], dt.bfloat16)
            # v in natural layout; per head 65 columns: 64 of v plus a ones
            # column that makes the attn@v matmul also emit the softmax denom.
            v_sb = proj_out.tile([P, JT, PAIRS * 2, DK + 1], dt.bfloat16)
            nc.vector.memset(v_sb[:, :, :, DK : DK + 1], 1.0)
            outt_sb = proj_out.tile([P, PAIRS, S], dt.bfloat16)

            vt_r = vt_d.rearrange("(c p) s -> p c s", p=P)
            kt_r = kt_d.rearrange("(c p) s -> p c s", p=P)
            qt_r = qt_d.rearrange("(c p) s -> p c s", p=P)

            def proj_step(dst_sb, lhs_chunks, rhs_chunks, bias_lhs, bias_rhs):
                """One [128, 512] projection psum: accumulate over C chunks."""
                ps = mm_ps.tile([P, 512], dt.float32, tag="ps", name="ps")
                for c in range(C):
                    nc.tensor.matmul(
                        ps[:], lhs_chunks(c), rhs_chunks(c),
                        start=(c == 0), stop=(c == C - 1 and not has_bq),
                    )
                if has_bq:
                    nc.tensor.matmul(ps[:], bias_lhs(), bias_rhs(),
                                     start=False, stop=True)
                return ps

            def qproj(i, d, qt_in):
                isl = slice(i * 512, (i + 1) * 512)
                dsl = slice(d * P, (d + 1) * P)
                ps = proj_step(
                    qT_sb,
                    lambda c: wq_sb[:, c, dsl],
                    lambda c, qt_in=qt_in: qt_in[:, c, :],
                    lambda: bq_sb[0:1, dsl],
                    lambda: ones_sb[0:1, :],
                )
                nc.vector.tensor_copy(out=qT_sb[:, d, isl], in_=ps[:])

            def fin(i, e):
                isl = slice(i * 512, (i + 1) * 512)
                esl = slice(e * P, (e + 1) * P)
                ps = mm_ps.tile([P, 512], dt.float32, tag="ps", name="ps")
                for hc in range(PAIRS):
                    nc.tensor.matmul(
                        ps[:], wo_sb[:, hc, esl], outt_sb[:, hc, isl],
                        start=(hc == 0), stop=(hc == PAIRS - 1),
                    )
                of = outsb.tile([P, 512], dt.float32, tag="of", name="of")
                nc.vector.tensor_copy(out=of[:], in_=ps[:])
                nc.sync.dma_start(out=out_d[esl, isl], in_=of[:])

            # ---- prologue DMAs, ordered so the first compute isn't gated
            # behind later transfers on the in-order sync queue ----
            kt_ins = [None] * IT
            kt_ins[0] = kin.tile([P, C, 512], dt.bfloat16, tag="kin", name="kt_in")
            nc.sync.dma_start(out=kt_ins[0][:], in_=kt_r[:, :, 0:512])
            qt_in0 = xin.tile([P, C, 512], dt.bfloat16, tag="qin", name="qt_in")
            nc.sync.dma_start(out=qt_in0[:], in_=qt_r[:, :, 0:512])
            qt_in_next = {0: qt_in0}
            vt_ins = {}
            for ic in (0, 1):
                vt_ins[ic] = xin.tile([P, C, 512], dt.bfloat16, tag="vin", bufs=3, name="vt_in")
                nc.sync.dma_start(
                    out=vt_ins[ic][:], in_=vt_r[:, :, ic * 512 : (ic + 1) * 512]
                )
            wo_sb = consts.tile([P, PAIRS, D], dt.bfloat16)
            nc.sync.dma_start(out=wo_sb[:], in_=wo_d.rearrange("(c p) e -> p c e", p=P))
            for ii in range(1, IT):
                kt_ins[ii] = kin.tile([P, C, 512], dt.bfloat16, tag="kin", name="kt_in")
                nc.sync.dma_start(
                    out=kt_ins[ii][:], in_=kt_r[:, :, ii * 512 : (ii + 1) * 512]
                )

            def kproj(d, ii):
                isl = slice(ii * 512, (ii + 1) * 512)
                dsl = slice(d * P, (d + 1) * P)
                ps = proj_step(
                    kT_sb,
                    lambda c: wq_sb[:, c, dsl],
                    lambda c: kt_ins[ii][:, c, :],
                    lambda: bq_sb[0:1, dsl],
                    lambda: ones_sb[0:1, :],
                )
                nc.vector.tensor_copy(out=kT_sb[:, d, isl], in_=ps[:])

            def qproj_filler(i, d):
                def _f():
                    if i not in qt_in_next:
                        t = xin.tile([P, C, 512], dt.bfloat16, tag="qin", name="qt_in")
                        nc.sync.dma_start(
                            out=t[:], in_=qt_r[:, :, i * 512 : (i + 1) * 512]
                        )
                        qt_in_next[i] = t
                    qproj(i, d, qt_in_next[i])
                return _f

            # only kT/qT (chunk 0, slice 0) before attention starts; the rest
            # rides as filler work inside the attention j-loops
            kproj(0, 0)
            qproj(0, 0, qt_in0)

            # filler queues: work to sprinkle into PE slack of the ACT-bound
            # attention j-loops. The ordering guarantees kT chunk 0's slices
            # land just ahead of pair 0's j progression, and chunk/qT pr+1
            # completes before pair pr+1 starts.
            fillers = {0: [], 1: [], 2: [], 3: []}
            for ii in range(1, IT):
                fillers[0].append(lambda ii=ii: kproj(0, ii))
            for d in range(1, PAIRS):
                fillers[0].append(qproj_filler(0, d))
                for ii in range(IT):
                    fillers[0].append(lambda d=d, ii=ii: kproj(d, ii))
            for d in range(PAIRS):
                fillers[0].append(qproj_filler(1, d))
            for i in (1, 2, 3):
                for e in range(ET):
                    fillers[i].append(lambda i=i, e=e: fin(i - 1, e))
                if i < IT - 1:
                    for d in range(PAIRS):
                        fillers[i].append(qproj_filler(i + 1, d))

            # ---- main loop over i tiles ----
            for i in range(IT):
                isl = slice(i * 512, (i + 1) * 512)
                for pr in range(PAIRS):
                    filler_js = (
                        (1, 3, 5, 7, 9, 11, 13, 15)
                        if (i == 0 and pr == 0)
                        else (3, 6, 9, 12, 15)
                    )
                    oa = o_ps.tile([DK + 1, 512], dt.float32, tag="oa", name="oa")
                    ob = o_ps.tile([DK + 1, 512], dt.float32, tag="ob", name="ob")
                    for j in range(JT):
                        jsl = slice(j * P, (j + 1) * P)
                        if i == 0 and pr == 0:
                            # v projection rides inside the first j-loop;
                            # slices 0/1 were prefetched in the prologue
                            if j in (4, 8):
                                ic = j // 4 + 1  # prefetch one slice ahead
                                vt_ins[ic] = xin.tile(
                                    [P, C, 512], dt.bfloat16, tag="vin", bufs=3,
                                    name="vt_in",
                                )
                                nc.sync.dma_start(
                                    out=vt_ins[ic][:],
                                    in_=vt_r[:, :, ic * 512 : (ic + 1) * 512],
                                )
                            vt_in = vt_ins[j // 4]
                            jl = j % 4
                            ps = proj_step(
                                v_sb,
                                lambda c, vt_in=vt_in, jl=jl: vt_in[:, c, jl * P : (jl + 1) * P],
                                lambda c: wq_sb[:, c, :],
                                lambda: ones_sb[0:1, 0:P],
                                lambda: bq_sb[0:1, :],
                            )
                            nc.vector.tensor_copy(
                                out=v_sb[:, j, :, 0:DK],
                                in_=ps[:].rearrange("p (h k) -> p h k", h=PAIRS * 2),
                            )
                        sc = sc_ps.tile([P, 2, 512], dt.float32, tag="sc", name="sc")
                        # two heads (K=64 each) packed into the PE array
                        nc.tensor.matmul(
                            sc[:, 0, :], kT_sb[0:DK, pr, jsl], qT_sb[0:DK, pr, isl],
                            start=True, stop=True,
                        )
                        nc.tensor.matmul(
                            sc[:, 1, :], kT_sb[DK:P, pr, jsl], qT_sb[DK:P, pr, isl],
                            start=True, stop=True,
                        )
                        ex = expp.tile([P, 2, 512], dt.bfloat16, tag="ex", name="ex")
                        nc.scalar.activation(
                            out=ex[:], in_=sc[:], func=AF.Exp, scale=0.125
                        )
                        nc.tensor.matmul(
                            oa[:], v_sb[:, j, 2 * pr, :], ex[:, 0, :],
                            start=(j == 0), stop=(j == JT - 1),
                        )
                        nc.tensor.matmul(
                            ob[:], v_sb[:, j, 2 * pr + 1, :], ex[:, 1, :],
                            start=(j == 0), stop=(j == JT - 1),
                        )
                        if j in filler_js and fillers[i]:
                            fillers[i].pop(0)()
                    # normalize: row DK of each psum is the softmax denominator.
                    # Single f32 copy frees the PSUM bank; the reciprocal chain
                    # then runs off the PE critical path.
                    for o_ps_t, upper in ((oa, False), (ob, True)):
                        o_sb = osbp.tile([DK + 1, 512], dt.float32, tag="osb", name="o_sb")
                        nc.vector.tensor_copy(out=o_sb[:], in_=o_ps_t[:])
                        den0 = small.tile([1, 512], dt.float32, tag="den0", name="den0")
                        nc.gpsimd.dma_start(out=den0[:], in_=o_sb[DK : DK + 1, :])
                        bc = small.tile([DK, 512], dt.float32, tag="bc", name="bc")
                        nc.gpsimd.partition_broadcast(bc[:], den0[:])
                        rc = small.tile([DK, 512], dt.float32, tag="rc", name="rc")
                        nc.vector.reciprocal_approx_fast(out=rc[:], in_=bc[:])
                        if not upper:
                            nc.vector.tensor_mul(
                                out=outt_sb[0:DK, pr, isl],
                                in0=o_sb[0:DK, :],
                                in1=rc[:],
                            )
                        else:
                            tmp = small.tile([DK, 512], dt.bfloat16, tag="tmp", name="tmp")
                            nc.vector.tensor_mul(
                                out=tmp[:], in0=o_sb[0:DK, :], in1=rc[:]
                            )
                            nc.gpsimd.dma_start(out=outt_sb[DK:P, pr, isl], in_=tmp[:])

                # drain any fillers not consumed inside the j-loops
                while fillers[i]:
                    fillers[i].pop(0)()

            # ---- epilogue: output projection of the last i tile ----
            for e in range(ET):
                fin(IT - 1, e)

    nc.compile()
    return nc


def _get_program(has_bq: bool):
    key = ("prog", has_bq)
    if key not in _CACHE:
        _CACHE[key] = _build(has_bq)
    return _CACHE[key]


def _run(inputs, trace=False):
    from concourse.bass_utils import run_bass_kernel_spmd

    Q = np.asarray(inputs["Q"], dtype=np.float32)
    K = np.asarray(inputs["K"], dtype=np.float32)
    V = np.asarray(inputs["V"], dtype=np.float32)
    Wq = np.asarray(inputs["Wq"], dtype=np.float32)
    bq = np.asarray(inputs["bq"], dtype=np.float32)
    Wo = np.asarray(inputs["Wo"], dtype=np.float32)
    bo = np.asarray(inputs["bo"], dtype=np.float32)

    has_bq = bool(np.any(bq))
    nc = _get_program(has_bq)

    qt = [np.ascontiguousarray(Q[b].T).astype(BF16) for b in range(B)]
    kt = [np.ascontiguousarray(K[b].T).astype(BF16) for b in range(B)]
    vt = [np.ascontiguousarray(V[b].T).astype(BF16) for b in range(B)]
    wq = [np.ascontiguousarray(Wq[:, g * DH : (g + 1) * DH]).astype(BF16) for g in range(2)]
    wo = [np.ascontiguousarray(Wo[g * DH : (g + 1) * DH, :]).astype(BF16) for g in range(2)]
    bqs = [bq[None, g * DH : (g + 1) * DH].astype(BF16) for g in range(2)]

    in_maps = []
    for b in range(B):
        for g in range(2):
            m = {"qt": qt[b], "kt": kt[b], "vt": vt[b], "wq": wq[g], "wo": wo[g]}
            if has_bq:
                m["bq"] = bqs[g]
            in_maps.append(m)

    res = run_bass_kernel_spmd(nc, in_maps, core_ids=list(range(8)), trace=trace)

    out = np.empty((B, S, D), np.float32)
    for b in range(B):
        part = res.results[2 * b]["out"] + res.results[2 * b + 1]["out"]
        out[b] = part.T + bo
    return out, res


def kernel(**inputs) -> np.ndarray:
    return _run(inputs, trace=False)[0]


def _ensure_ntff_hook():
    """The agent image's antenv lacks axon_hooks; synthesize it so
    run_bass_kernel_spmd(trace=True) can capture NTFF profiles."""
    import sys, types
    try:
        from antenv.axon_hooks import get_axon_ntff_profile_hook  # noqa: F401
        return
    except ImportError:
        pass
    mod = types.ModuleType("antenv.axon_hooks")
    mod._hook = None
    mod.set_axon_ntff_profile_hook = lambda h: setattr(mod, "_hook", h)
    mod.get_axon_ntff_profile_hook = lambda: mod._hook
    sys.modules["antenv.axon_hooks"] = mod
    import antenv
    antenv.axon_hooks = mod
    try:
        from trn_agent_boot.trn_boot import _ntff_profile_via_ctypes
        mod._hook = _ntff_profile_via_ctypes("/opt/axon/libaxon_pjrt.so")
    except Exception as e:  # degrade: trace skipped, run still works
        print(f"ntff hook install failed: {e}")


def kernel_traced(**inputs):
    _ensure_ntff_hook()
    return _run(inputs, trace=True)


# revision 21
# speedup vs baseline: 1.0600x; 1.0561x over previous
"""Multi-head attention (B=4, S=2048, D=1024, H=16, dk=64) on 8 TRN2 NeuronCores.

Sharding: core = (batch b, head-group g) with b in 0..3, g in 0..1.
Each core projects q/k/v for its batch with its 512 Wq columns (8 heads),
runs attention for those heads, and applies its 512 Wo rows, producing a
partial (transposed) output. Host sums the two half-head partials per batch.

All device compute in bf16 (fp32 PSUM accumulation). Layout is transposed
(feature-major) so every matmul has its contraction dim on partitions:
  qT,kT: [dh, S]   v: [S, dh(+ones col)]   scores^T: [j, i]   out^T: [e, i]

Per head pair, the two K=64 score matmuls are packed into the PE array via
base-partition-derived tile_position (rows 0-63 / 64-127 concurrently).
The attn@v matmul carries a 65th ones-column of v so each PSUM tile's last
partition accumulates the softmax denominator for free.

Software pipelining: kT is projected chunk-wise so attention i=0 starts
early; v is projected inside (i=0, pair=0)'s j-loop; the output projection
of i-1 and the q projection of i+1 are spread between pair blocks so the
scalar engine (exp) stays saturated across i boundaries.
"""

import numpy as np
import ml_dtypes

B, S, D = 4, 2048, 1024
N_HEAD, DK = 16, 64
P = 128
DH = 512          # head-group width (8 heads)
PAIRS = 4         # head pairs per core
C = D // P        # contraction chunks for projections (8)
IT = S // 512     # i tiles of 512 (4)
JT = S // P       # j tiles of 128 (16)
ET = D // P       # output-row tiles (8)

BF16 = ml_dtypes.bfloat16

_CACHE = {}


def _build(has_bq: bool):
    import concourse.bacc as bacc
    import concourse.mybir as mybir
    import concourse.tile as tile

    nc = bacc.Bacc()
    dt = mybir.dt
    AF = mybir.ActivationFunctionType

    qt_d = nc.dram_tensor("qt", [D, S], dt.bfloat16, kind="ExternalInput")
    kt_d = nc.dram_tensor("kt", [D, S], dt.bfloat16, kind="ExternalInput")
    vt_d = nc.dram_tensor("vt", [D, S], dt.bfloat16, kind="ExternalInput")
    wq_d = nc.dram_tensor("wq", [D, DH], dt.bfloat16, kind="ExternalInput")
    wo_d = nc.dram_tensor("wo", [DH, D], dt.bfloat16, kind="ExternalInput")
    if has_bq:
        bq_d = nc.dram_tensor("bq", [1, DH], dt.bfloat16, kind="ExternalInput")
    out_d = nc.dram_tensor("out", [D, S], dt.float32, kind="ExternalOutput")

    with tile.TileContext(nc) as tc:
        with (
            tc.tile_pool(name="consts", bufs=1) as consts,
            tc.tile_pool(name="xin", bufs=2) as xin,
            tc.tile_pool(name="kin", bufs=4) as kin,
            tc.tile_pool(name="proj_out", bufs=1) as proj_out,
            tc.tile_pool(name="expp", bufs=7) as expp,
            tc.tile_pool(name="osb", bufs=3) as osbp,
            tc.tile_pool(name="small", bufs=3) as small,
            tc.tile_pool(name="outsb", bufs=2) as outsb,
            tc.tile_pool(name="mm_ps", bufs=2, space="PSUM") as mm_ps,
            tc.tile_pool(name="sc_ps", bufs=2, space="PSUM") as sc_ps,
            tc.tile_pool(name="o_ps", bufs=1, space="PSUM") as o_ps,
        ):
            wq_sb = consts.tile([P, C, DH], dt.bfloat16)
            nc.sync.dma_start(out=wq_sb[:], in_=wq_d.rearrange("(c p) d -> p c d", p=P))
            if has_bq:
                bq_sb = consts.tile([1, DH], dt.bfloat16)
                nc.sync.dma_start(out=bq_sb[:], in_=bq_d[:])
                ones_sb = consts.tile([1, 512], dt.bfloat16)
                nc.vector.memset(ones_sb[:], 1.0)

            qT_sb = proj_out.tile([P, PAIRS, S], dt.bfloat16)
            kT_sb = proj_out.tile([P, PAIRS, S], dt.bfloat16)
            # v in natural layout; per head 65 columns: 64 of v plus a ones
            # column that makes the attn@v matmul also emit the softmax denom.
            v_sb = proj_out.tile([P, JT, PAIRS * 2, DK + 1], dt.bfloat16)
            nc.vector.memset(v_sb[:, :, :, DK : DK + 1], 1.0)
            outt_sb = proj_out.tile([P, PAIRS, S], dt.bfloat16)

            vt_r = vt_d.rearrange("(c p) s -> p c s", p=P)
            kt_r = kt_d.rearrange("(c p) s -> p c s", p=P)
            qt_r = qt_d.rearrange("(c p) s -> p c s", p=P)

            def proj_step(dst_sb, lhs_chunks, rhs_chunks, bias_lhs, bias_rhs):
                """One [128, 512] projection psum: accumulate over C chunks."""
                ps = mm_ps.tile([P, 512], dt.float32, tag="ps", name="ps")
                for c in range(C):
                    nc.tensor.matmul(
                        ps[:], lhs_chunks(c), rhs_chunks(c),
                        start=(c == 0), stop=(c == C - 1 and not has_bq),
                    )
                if has_bq:
                    nc.tensor.matmul(ps[:], bias_lhs(), bias_rhs(),
                                     start=False, stop=True)
                return ps

            def qproj(i, d, qt_in):
                isl = slice(i * 512, (i + 1) * 512)
                dsl = slice(d * P, (d + 1) * P)
                ps = proj_step(
                    qT_sb,
                    lambda c: wq_sb[:, c, dsl],
                    lambda c, qt_in=qt_in: qt_in[:, c, :],
                    lambda: bq_sb[0:1, dsl],
                    lambda: ones_sb[0:1, :],
                )
                nc.vector.tensor_copy(out=qT_sb[:, d, isl], in_=ps[:])

            def fin(i, e):
                isl = slice(i * 512, (i + 1) * 512)
                esl = slice(e * P, (e + 1) * P)
                ps = mm_ps.tile([P, 512], dt.float32, tag="ps", name="ps")
                for hc in range(PAIRS):
                    nc.tensor.matmul(
                        ps[:], wo_sb[:, hc, esl], outt_sb[:, hc, isl],
                        start=(hc == 0), stop=(hc == PAIRS - 1),
                    )
                of = outsb.tile([P, 512], dt.float32, tag="of", name="of")
                nc.vector.tensor_copy(out=of[:], in_=ps[:])
                nc.sync.dma_start(out=out_d[esl, isl], in_=of[:])

            # ---- prologue DMAs, ordered so the first compute isn't gated
            # behind later transfers on the in-order sync queue ----
            kt_ins = [None] * IT
            kt_ins[0] = kin.tile([P, C, 512], dt.bfloat16, tag="kin", name="kt_in")
            nc.sync.dma_start(out=kt_ins[0][:], in_=kt_r[:, :, 0:512])
            qt_in0 = xin.tile([P, C, 512], dt.bfloat16, tag="qin", name="qt_in")
            nc.sync.dma_start(out=qt_in0[:], in_=qt_r[:, :, 0:512])
            qt_in_next = {0: qt_in0}
            vt_ins = {}
            for ic in (0, 1):
                vt_ins[ic] = xin.tile([P, C, 512], dt.bfloat16, tag="vin", bufs=2, name="vt_in")
                nc.sync.dma_start(
                    out=vt_ins[ic][:], in_=vt_r[:, :, ic * 512 : (ic + 1) * 512]
                )
            wo_sb = consts.tile([P, PAIRS, D], dt.bfloat16)
            nc.sync.dma_start(out=wo_sb[:], in_=wo_d.rearrange("(c p) e -> p c e", p=P))
            for ii in range(1, IT):
                kt_ins[ii] = kin.tile([P, C, 512], dt.bfloat16, tag="kin", name="kt_in")
                nc.sync.dma_start(
                    out=kt_ins[ii][:], in_=kt_r[:, :, ii * 512 : (ii + 1) * 512]
                )

            def kproj(d, ii):
                isl = slice(ii * 512, (ii + 1) * 512)
                dsl = slice(d * P, (d + 1) * P)
                ps = proj_step(
                    kT_sb,
                    lambda c: wq_sb[:, c, dsl],
                    lambda c: kt_ins[ii][:, c, :],
                    lambda: bq_sb[0:1, dsl],
                    lambda: ones_sb[0:1, :],
                )
                nc.vector.tensor_copy(out=kT_sb[:, d, isl], in_=ps[:])

            def qproj_filler(i, d):
                def _f():
                    if i not in qt_in_next:
                        t = xin.tile([P, C, 512], dt.bfloat16, tag="qin", name="qt_in")
                        nc.sync.dma_start(
                            out=t[:], in_=qt_r[:, :, i * 512 : (i + 1) * 512]
                        )
                        qt_in_next[i] = t
                    qproj(i, d, qt_in_next[i])
                return _f

            # only kT/qT (chunk 0, slice 0) before attention starts; the rest
            # rides as filler work inside the attention j-loops
            kproj(0, 0)
            qproj(0, 0, qt_in0)

            # filler queues: work to sprinkle into PE slack of the ACT-bound
            # attention j-loops. The ordering guarantees kT chunk 0's slices
            # land just ahead of pair 0's j progression, and chunk/qT pr+1
            # completes before pair pr+1 starts.
            fillers = {0: [], 1: [], 2: [], 3: []}
            for ii in range(1, IT):
                fillers[0].append(lambda ii=ii: kproj(0, ii))
            for d in range(1, PAIRS):
                fillers[0].append(qproj_filler(0, d))
                for ii in range(IT):
                    fillers[0].append(lambda d=d, ii=ii: kproj(d, ii))
            for d in range(PAIRS):
                fillers[0].append(qproj_filler(1, d))
            for i in (1, 2, 3):
                # qproj first: fin(i-1) must trail pair 3's normalize chain
                if i < IT - 1:
                    for d in range(PAIRS):
                        fillers[i].append(qproj_filler(i + 1, d))
                for e in range(ET):
                    fillers[i].append(lambda i=i, e=e: fin(i - 1, e))

            def vproj(j):
                """v projection for j-tile j; rides inside block 0's j-loop."""
                if j in (4, 8):
                    ic = j // 4 + 1  # prefetch one slice ahead
                    vt_ins[ic] = xin.tile(
                        [P, C, 512], dt.bfloat16, tag="vin", bufs=2, name="vt_in"
                    )
                    nc.sync.dma_start(
                        out=vt_ins[ic][:],
                        in_=vt_r[:, :, ic * 512 : (ic + 1) * 512],
                    )
                vt_in = vt_ins[j // 4]
                jl = j % 4
                ps = proj_step(
                    v_sb,
                    lambda c: vt_in[:, c, jl * P : (jl + 1) * P],
                    lambda c: wq_sb[:, c, :],
                    lambda: ones_sb[0:1, 0:P],
                    lambda: bq_sb[0:1, :],
                )
                nc.vector.tensor_copy(
                    out=v_sb[:, j, :, 0:DK],
                    in_=ps[:].rearrange("p (h k) -> p h k", h=PAIRS * 2),
                )

            def normalize(pr, isl, oa, ob):
                # row DK of each psum is the softmax denominator. Single f32
                # copy frees the PSUM bank; the reciprocal chain then runs
                # off the PE critical path.
                for o_ps_t, upper in ((oa, False), (ob, True)):
                    o_sb = osbp.tile([DK + 1, 512], dt.float32, tag="osb", name="o_sb")
                    nc.vector.tensor_copy(out=o_sb[:], in_=o_ps_t[:])
                    den0 = small.tile([1, 512], dt.float32, tag="den0", name="den0")
                    nc.gpsimd.dma_start(out=den0[:], in_=o_sb[DK : DK + 1, :])
                    bc = small.tile([DK, 512], dt.float32, tag="bc", name="bc")
                    nc.gpsimd.partition_broadcast(bc[:], den0[:])
                    rc = small.tile([DK, 512], dt.float32, tag="rc", name="rc")
                    nc.vector.reciprocal_approx_fast(out=rc[:], in_=bc[:])
                    if not upper:
                        nc.vector.tensor_mul(
                            out=outt_sb[0:DK, pr, isl],
                            in0=o_sb[0:DK, :],
                            in1=rc[:],
                        )
                    else:
                        tmp = small.tile([DK, 512], dt.bfloat16, tag="tmp", name="tmp")
                        nc.vector.tensor_mul(out=tmp[:], in0=o_sb[0:DK, :], in1=rc[:])
                        nc.gpsimd.dma_start(out=outt_sb[DK:P, pr, isl], in_=tmp[:])

            # ---- main attention stream, software-pipelined with LAG steps
            # between exp production (scores -> ACT) and consumption (attn@v).
            # This keeps the scalar engine saturated while the PE spends its
            # slack on attn@v, projections, and output-projection fillers. ----
            LAG = 4
            blocks = [(i, pr) for i in range(IT) for pr in range(PAIRS)]
            ex_tiles = {}
            o_tiles = {}
            n_steps = JT * len(blocks) + LAG
            for s in range(n_steps):
                t, j = divmod(s, JT)
                if t < len(blocks):
                    i, pr = blocks[t]
                    isl = slice(i * 512, (i + 1) * 512)
                    jsl = slice(j * P, (j + 1) * P)
                    if t == 0:
                        vproj(j)
                    sc = sc_ps.tile([P, 2, 512], dt.float32, tag="sc", name="sc")
                    # two heads (K=64 each) packed into the PE array
                    nc.tensor.matmul(
                        sc[:, 0, :], kT_sb[0:DK, pr, jsl], qT_sb[0:DK, pr, isl],
                        start=True, stop=True,
                    )
                    nc.tensor.matmul(
                        sc[:, 1, :], kT_sb[DK:P, pr, jsl], qT_sb[DK:P, pr, isl],
                        start=True, stop=True,
                    )
                    ex = expp.tile([P, 2, 512], dt.bfloat16, tag="ex", name="ex")
                    nc.scalar.activation(out=ex[:], in_=sc[:], func=AF.Exp, scale=0.125)
                    ex_tiles[s] = ex
                sp = s - LAG
                if sp >= 0:
                    tp, jp = divmod(sp, JT)
                    ip, prp = blocks[tp]
                    if jp == 0:
                        o_tiles[tp] = (
                            o_ps.tile([DK + 1, 512], dt.float32, tag="oa", name="oa"),
                            o_ps.tile([DK + 1, 512], dt.float32, tag="ob", name="ob"),
                        )
                    oa, ob = o_tiles[tp]
                    ex = ex_tiles.pop(sp)
                    nc.tensor.matmul(
                        oa[:], v_sb[:, jp, 2 * prp, :], ex[:, 0, :],
                        start=(jp == 0), stop=(jp == JT - 1),
                    )
                    nc.tensor.matmul(
                        ob[:], v_sb[:, jp, 2 * prp + 1, :], ex[:, 1, :],
                        start=(jp == 0), stop=(jp == JT - 1),
                    )
                    if jp == JT - 1:
                        normalize(prp, slice(ip * 512, (ip + 1) * 512), oa, ob)
                        del o_tiles[tp]
                if t < len(blocks):
                    i = blocks[t][0]
                    filler_js = (
                        (1, 3, 5, 7, 9, 11, 13, 15) if t == 0 else (3, 6, 9, 12, 15)
                    )
                    if j in filler_js and fillers[i]:
                        fillers[i].pop(0)()
                    if j == JT - 1 and t % PAIRS == PAIRS - 1:
                        # end of an i group: drain its remaining fillers
                        while fillers[i]:
                            fillers[i].pop(0)()

            # ---- epilogue: output projection of the last i tile ----
            for e in range(ET):
                fin(IT - 1, e)

    nc.compile()
    return nc


def _get_program(has_bq: bool):
    key = ("prog", has_bq)
    if key not in _CACHE:
        _CACHE[key] = _build(has_bq)
    return _CACHE[key]


def _run(inputs, trace=False):
    from concourse.bass_utils import run_bass_kernel_spmd

    Q = np.asarray(inputs["Q"], dtype=np.float32)
    K = np.asarray(inputs["K"], dtype=np.float32)
    V = np.asarray(inputs["V"], dtype=np.float32)
    Wq = np.asarray(inputs["Wq"], dtype=np.float32)
    bq = np.asarray(inputs["bq"], dtype=np.float32)
    Wo = np.asarray(inputs["Wo"], dtype=np.float32)
    bo = np.asarray(inputs["bo"], dtype=np.float32)

    has_bq = bool(np.any(bq))
    nc = _get_program(has_bq)

    qt = [np.ascontiguousarray(Q[b].T).astype(BF16) for b in range(B)]
    kt = [np.ascontiguousarray(K[b].T).astype(BF16) for b in range(B)]
    vt = [np.ascontiguousarray(V[b].T).astype(BF16) for b in range(B)]
    wq = [np.ascontiguousarray(Wq[:, g * DH : (g + 1) * DH]).astype(BF16) for g in range(2)]
    wo = [np.ascontiguousarray(Wo[g * DH : (g + 1) * DH, :]).astype(BF16) for g in range(2)]
    bqs = [bq[None, g * DH : (g + 1) * DH].astype(BF16) for g in range(2)]

    in_maps = []
    for b in range(B):
        for g in range(2):
            m = {"qt": qt[b], "kt": kt[b], "vt": vt[b], "wq": wq[g], "wo": wo[g]}
            if has_bq:
                m["bq"] = bqs[g]
            in_maps.append(m)

    res = run_bass_kernel_spmd(nc, in_maps, core_ids=list(range(8)), trace=trace)

    out = np.empty((B, S, D), np.float32)
    for b in range(B):
        part = res.results[2 * b]["out"] + res.results[2 * b + 1]["out"]
        out[b] = part.T + bo
    return out, res


def kernel(**inputs) -> np.ndarray:
    return _run(inputs, trace=False)[0]


def _ensure_ntff_hook():
    """The agent image's antenv lacks axon_hooks; synthesize it so
    run_bass_kernel_spmd(trace=True) can capture NTFF profiles."""
    import sys, types
    try:
        from antenv.axon_hooks import get_axon_ntff_profile_hook  # noqa: F401
        return
    except ImportError:
        pass
    mod = types.ModuleType("antenv.axon_hooks")
    mod._hook = None
    mod.set_axon_ntff_profile_hook = lambda h: setattr(mod, "_hook", h)
    mod.get_axon_ntff_profile_hook = lambda: mod._hook
    sys.modules["antenv.axon_hooks"] = mod
    import antenv
    antenv.axon_hooks = mod
    try:
        from trn_agent_boot.trn_boot import _ntff_profile_via_ctypes
        mod._hook = _ntff_profile_via_ctypes("/opt/axon/libaxon_pjrt.so")
    except Exception as e:  # degrade: trace skipped, run still works
        print(f"ntff hook install failed: {e}")


def kernel_traced(**inputs):
    _ensure_ntff_hook()
    return _run(inputs, trace=True)
